# revision 4
# baseline (speedup 1.0000x reference)
"""Int8-quantized linear: y = x @ (w_q * scale)^T + bias, tensor-parallel on 8 cores.

Shapes (hardcoded): x [4,32,4096] f32, w_q [11008,4096] int8, scale [1] f32,
bias [11008] f32 -> out [4,32,11008] f32.

Column-parallel over out_features (1376 per core), raw Bass (no TileContext):
hand-rolled semaphores keep the prologue, per-chunk sync overhead, and the
end-of-kernel teardown battery minimal. Each core streams its int8 shard over
the Sync HWDGE ring (host pre-swizzled to SBUF layout [p, k_chunk, n]),
upconverts int8->fp16 split across DVE(512)/ACT(448)/GPSIMD(416) columns --
the splits match the three PSUM banks so every matmul carries exactly one
converter wait -- and accumulates 32 K-chunk fp16 matmuls per bank. A few
dummy matmuls at kernel start warm the PE clock (HAM) during the DMA ramp.
Bias enters PSUM via a K=2 ones-matmul (fp16 hi+lo). PSUM is evicted to fp16
by DVE/ACT and DMA'd out; the host upcasts and concatenates the 8 shards.
"""

import numpy as np

P = 128            # partitions = B*S tokens
IN_F = 4096
OUT_F = 11008
N_CORES = 8
N_SHARD = OUT_F // N_CORES          # 1376
K_CHUNKS = IN_F // P                # 32
CA, CB, CC = 512, 448, 416          # converter/psum-bank column split (DVE/ACT/GPS)
WGROUPS = [2, 2, 4, 4, 4, 4, 4, 4, 4]   # k-chunks per weight DMA
XSPLIT = 4                          # k-chunks in the first x DMA
N_WARM = 4                          # PE warm-up matmuls (N=512)
BIAS_AT = 16                        # bias matmuls run after this chunk's matmuls

_CACHE = {}


def _build_nc():
    import concourse.bass as bass
    import concourse.mybir as mybir

    fp16 = mybir.dt.float16
    nc = bass.Bass()
    xs_d = nc.declare_dram_parameter("xs", [P, IN_F], fp16, isOutput=False)
    wq_d = nc.declare_dram_parameter("wq", [P, K_CHUNKS, N_SHARD], mybir.dt.int8,
                                     isOutput=False)
    bi_d = nc.declare_dram_parameter("bias2", [2, N_SHARD], fp16, isOutput=False)
    out_d = nc.declare_dram_parameter("out", [P, N_SHARD], fp16, isOutput=True)

    xs = nc.alloc_sbuf_tensor("xs_sb", [P, IN_F], fp16)
    w8 = nc.alloc_sbuf_tensor("w8_sb", [P, K_CHUNKS, N_SHARD], mybir.dt.int8)
    w16 = nc.alloc_sbuf_tensor("w16_sb", [P, K_CHUNKS, N_SHARD], fp16)
    bias = nc.alloc_sbuf_tensor("bias_sb", [2, N_SHARD], fp16)
    warm = nc.alloc_sbuf_tensor("warm_sb", [2, 512], fp16)
    stage = nc.alloc_sbuf_tensor("stage_sb", [P, N_SHARD], fp16)

    ps0 = nc.alloc_psum_tensor("ps0", [P, CA], mybir.dt.float32)
    ps1 = nc.alloc_psum_tensor("ps1", [P, CB], mybir.dt.float32)
    ps2 = nc.alloc_psum_tensor("ps2", [P, CC], mybir.dt.float32)
    scr = nc.alloc_psum_tensor("scr", [P, 512], mybir.dt.float32)

    s_warm = nc.alloc_semaphore("s_warm")
    s_x = nc.alloc_semaphore("s_x")
    s_b = nc.alloc_semaphore("s_b")
    s_w = nc.alloc_semaphore("s_w")
    s_cva = nc.alloc_semaphore("s_cva")
    s_cvb = nc.alloc_semaphore("s_cvb")
    s_cvc = nc.alloc_semaphore("s_cvc")
    s_mm = nc.alloc_semaphore("s_mm")
    s_ev0 = nc.alloc_semaphore("s_ev0")
    s_ev1 = nc.alloc_semaphore("s_ev1")
    s_ev2 = nc.alloc_semaphore("s_ev2")
    s_out = nc.alloc_semaphore("s_out")
    all_sems = [s_warm, s_x, s_b, s_w, s_cva, s_cvb, s_cvc, s_mm,
                s_ev0, s_ev1, s_ev2, s_out]

    # ---- Sync: all HWDGE DMA triggers, in ring-FIFO order ----
    # x chunks 0..XSPLIT-1 first so the first matmul isn't gated on all of x.
    nc.sync.dma_start(out=xs[:, :XSPLIT * P], in_=xs_d[:, :XSPLIT * P]) \
        .then_inc(s_x, 16)
    k0 = 0
    for g, gsz in enumerate(WGROUPS):
        nc.sync.dma_start(out=w8[:, k0:k0 + gsz, :], in_=wq_d[:, k0:k0 + gsz, :]) \
            .then_inc(s_w, 16)
        if g == 0:
            nc.sync.dma_start(out=xs[:, XSPLIT * P:], in_=xs_d[:, XSPLIT * P:]) \
                .then_inc(s_x, 16)
            nc.sync.dma_start(out=bias[:, :], in_=bi_d[:, :]).then_inc(s_b, 16)
        k0 += gsz
    # output DMAs: each waits only on its own evictor
    nc.sync.dma_start(out=out_d[:, 0:CA], in_=stage[:, 0:CA]) \
        ._wait_ge(s_ev0, 1).then_inc(s_out, 16)
    nc.sync.dma_start(out=out_d[:, CA:CA + CB], in_=stage[:, CA:CA + CB]) \
        ._wait_ge(s_ev1, 1).then_inc(s_out, 16)
    nc.sync.dma_start(out=out_d[:, CA + CB:], in_=stage[:, CA + CB:]) \
        ._wait_ge(s_ev2, 1).then_inc(s_out, 16)

    # ---- Vector (DVE): warm-tile memset, conversions (cols 0:CA), evictions ----
    nc.vector.memset(warm[:, :], 1.0).then_inc(s_warm)
    k0 = 0
    for g, gsz in enumerate(WGROUPS):
        nc.vector.tensor_copy(w16[:, k0:k0 + gsz, 0:CA], w8[:, k0:k0 + gsz, 0:CA]) \
            ._wait_ge(s_w, 16 * (g + 1)).then_inc(s_cva)
        k0 += gsz
    nc.vector.tensor_copy(stage[:, 0:CA], ps0[:, :]) \
        ._wait_ge(s_mm, 1).then_inc(s_ev0)
    nc.vector.tensor_copy(stage[:, CA + CB:], ps2[:, :]) \
        ._wait_ge(s_mm, 3).then_inc(s_ev2)

    # ---- Scalar (ACT): conversions (cols CA:CA+CB), eviction of bank 1 ----
    k0 = 0
    for g, gsz in enumerate(WGROUPS):
        nc.scalar.copy(w16[:, k0:k0 + gsz, CA:CA + CB], w8[:, k0:k0 + gsz, CA:CA + CB]) \
            ._wait_ge(s_w, 16 * (g + 1)).then_inc(s_cvb)
        k0 += gsz
    nc.scalar.copy(stage[:, CA:CA + CB], ps1[:, :]) \
        ._wait_ge(s_mm, 2).then_inc(s_ev1)

    # ---- GpSimd: conversions (cols CA+CB:), final semaphore cleanup ----
    k0 = 0
    for g, gsz in enumerate(WGROUPS):
        nc.gpsimd.tensor_copy(w16[:, k0:k0 + gsz, CA + CB:], w8[:, k0:k0 + gsz, CA + CB:]) \
            ._wait_ge(s_w, 16 * (g + 1)).then_inc(s_cvc)
        k0 += gsz
    # reset sems to their initial (cleared) state so the NEFF can re-execute;
    # s_out>=48 transitively orders this after every other instruction
    sem_nums = sorted(h.num for h in all_sems)
    assert sem_nums == list(range(sem_nums[0], sem_nums[0] + len(sem_nums)))
    sem_range = range(sem_nums[0], sem_nums[-1] + 1)
    nc.gpsimd.wait_ge(s_out, 48)
    nc.gpsimd.dma_reset(sem_range)
    nc.gpsimd.sem_clear(sem_range)

    # ---- Tensor: warm-up, 32 x 3 matmuls, bias matmuls ----
    for i in range(N_WARM):
        nc.tensor.matmul(scr[:, :], lhsT=warm[:, 0:P], rhs=warm[:, :],
                         start=True, stop=True) \
            ._maybe_wait_ge((s_warm, 1) if i == 0 else None)
    grp = []                       # chunk -> weight-group index
    for g, gsz in enumerate(WGROUPS):
        grp += [g] * gsz
    banks = ((ps0, s_cva, 0, CA), (ps1, s_cvb, CA, CB), (ps2, s_cvc, CA + CB, CC))
    for k in range(K_CHUNKS):
        # InstMatmult takes at most one sync-wait, so the x-availability
        # waits ride as standalone sequencer waits
        if k == 0:
            nc.tensor.wait_ge(s_x, 16)
        elif k == XSPLIT:
            nc.tensor.wait_ge(s_x, 32)
        lhsT = xs[:, k * P:(k + 1) * P]
        last = k == K_CHUNKS - 1
        for ps, s_cv, lo, sz in banks:
            m = nc.tensor.matmul(ps[:, :], lhsT=lhsT, rhs=w16[:, k, lo:lo + sz],
                                 start=(k == 0), stop=last) \
                ._wait_ge(s_cv, grp[k] + 1)
            if last:
                m.then_inc(s_mm)
        if k == BIAS_AT:
            # bias mid-stream: psum[m, n] += 1*b_hi[n] + 1*b_lo[n]
            for j, (ps, _, lo, sz) in enumerate(banks):
                mb = nc.tensor.matmul(ps[:, :], lhsT=warm[:, 0:P],
                                      rhs=bias[:, lo:lo + sz],
                                      start=False, stop=False)
                if j == 0:
                    mb._wait_ge(s_b, 16)
    return nc


def get_nc():
    if "nc" not in _CACHE:
        _CACHE["nc"] = _build_nc()
    return _CACHE["nc"]


def make_in_maps(x, w_q, scale, bias):
    """Host-side shard/layout prep. Returns list of 8 per-core input dicts."""
    x = np.asarray(x, dtype=np.float32).reshape(P, IN_F)
    s = float(np.asarray(scale).reshape(-1)[0])
    xsc = (x * s).astype(np.float16)
    # SBUF layout: x_sb[p, nk*128+m] = xsc[m, nk*128+p] (contraction on partitions)
    x_sb = np.ascontiguousarray(
        xsc.reshape(P, K_CHUNKS, P).transpose(2, 1, 0)
    ).reshape(P, IN_F)

    w8 = np.asarray(w_q).astype(np.int8)
    wT = w8.T  # [IN_F, OUT_F]

    b32 = np.asarray(bias, dtype=np.float32)
    b_hi = b32.astype(np.float16)
    b_lo = (b32 - b_hi.astype(np.float32)).astype(np.float16)

    in_maps = []
    for c in range(N_CORES):
        lo, hi = c * N_SHARD, (c + 1) * N_SHARD
        shard = wT[:, lo:hi]                       # [IN_F, N_SHARD]
        # [p, k_chunk, n] so each DMA group is per-partition contiguous
        w_dma = np.ascontiguousarray(
            shard.reshape(K_CHUNKS, P, N_SHARD).transpose(1, 0, 2))
        in_maps.append({
            "xs": x_sb,
            "wq": w_dma,
            "bias2": np.ascontiguousarray(
                np.stack([b_hi[lo:hi], b_lo[lo:hi]], axis=0)
            ),
        })
    return in_maps


def gather(results):
    """results: list of 8 dicts with 'out' [P, N_SHARD] fp16 -> full output."""
    full = np.concatenate(
        [np.asarray(r["out"]).astype(np.float32) for r in results], axis=1)
    return np.ascontiguousarray(full.reshape(4, 32, OUT_F))


def kernel(x, w_q, scale, bias):
    from concourse.bass_utils import run_bass_kernel_spmd

    nc = get_nc()
    in_maps = make_in_maps(x, w_q, scale, bias)
    res = run_bass_kernel_spmd(nc, in_maps, list(range(N_CORES)))
    return gather(res.results)
